# revision 1
# baseline (speedup 1.0000x reference)
"""CrossSharedUnit Trainium2 kernel — 8-core data-parallel over batch.

Reference computation (per batch b, S=128 tokens, H=512 hidden, K=8):
  proj[b,s,k,g] = sum_h left[b,s,h] * G[h,k,g]
  raw[b,s,t,k]  = tanh(sum_g proj[b,s,k,g] * right[b,t,g])
  score[b,s,t]  = sum_k raw[b,s,t,k] * v[k]
  attn          = softmax(score, axis=t)
  out           = self + attn @ other_hidden
for two branches (aspect: left=aspect, right=polarity; polarity: left=aspect,
right=aspect — faithful to the source which uses aspect on both sides).

Sharding: batch B=32 split 4-per-core across 8 cores; G tensors replicated.
No collectives. Activations are shipped both natural [bs,h] and pre-transposed
[h,bs] from the host so every matmul contraction lands on the partition axis.

All matmuls run as float32r (TF32-like, 1 cycle/row at N>=512 vs 4 for fp32;
measured ~4e-4 rel err). Softmax needs no max-subtraction: |score| <= sum|v_k|
so exp() cannot overflow in fp32. The softmax division is deferred through the
attention matmul: out = self + (E @ other) / Z with Z from a ones-matmul,
applied as a per-partition scalar in the final fused DVE op.
"""

import os
import sys

sys.path.insert(0, "/opt/trn_rl_repo")

import numpy as np

from concourse import bacc, mybir, tile
from concourse.bass_utils import run_bass_kernel_spmd

B, S, H, K = 32, 128, 512, 8
NCORES = 8
BL = B // NCORES          # batches per core
BS = BL * S               # rows per core (512)
P = 128                   # partitions
HT = H // P               # h partition-tiles (4)
KG = K * H                # flattened (k,g) axis (4096)
F32 = mybir.dt.float32
F32R = mybir.dt.float32r

_cache = {}


def _build():
    """Build + compile the per-core Bass program (same program on all cores)."""
    nc = bacc.Bacc("TRN2", target_bir_lowering=False, debug=False,
                   num_devices=NCORES)

    xa_nat_d = nc.dram_tensor("xa_nat", [BS, H], F32R, kind="ExternalInput")
    xp_nat_d = nc.dram_tensor("xp_nat", [BS, H], F32R, kind="ExternalInput")
    xa_t_d = nc.dram_tensor("xa_t", [H, BS], F32R, kind="ExternalInput")
    xp_t_d = nc.dram_tensor("xp_t", [H, BS], F32R, kind="ExternalInput")
    g_ap_d = nc.dram_tensor("g_ap", [H, KG], F32R, kind="ExternalInput")
    g_pa_d = nc.dram_tensor("g_pa", [H, KG], F32R, kind="ExternalInput")
    v_ap_d = nc.dram_tensor("v_ap", [K, 1], F32, kind="ExternalInput")
    v_pa_d = nc.dram_tensor("v_pa", [K, 1], F32, kind="ExternalInput")
    out_a_d = nc.dram_tensor("out_a", [BS, H], F32, kind="ExternalOutput")
    out_p_d = nc.dram_tensor("out_p", [BS, H], F32, kind="ExternalOutput")

    with tile.TileContext(nc) as tc:
        with (
            tc.tile_pool(name="const", bufs=1) as cpool,
            tc.tile_pool(name="gk0p", bufs=2) as gk0p,
            tc.tile_pool(name="ghbp", bufs=1) as ghbp,
            tc.tile_pool(name="proj", bufs=1) as projpool,
            tc.tile_pool(name="work", bufs=3) as work,
            tc.tile_pool(name="ps_mm", bufs=3, space="PSUM") as ps_mm,
            tc.tile_pool(name="ps_z", bufs=1, space="PSUM") as ps_z,
            tc.tile_pool(name="ps_o", bufs=2, space="PSUM") as ps_o,
        ):
            # ---- persistent activations -------------------------------
            xa_nat = [cpool.tile([P, H], F32R, tag=f"xa_nat{i}", name=f"xa_nat{i}") for i in range(BL)]
            xp_nat = [cpool.tile([P, H], F32R, tag=f"xp_nat{i}", name=f"xp_nat{i}") for i in range(BL)]
            xa_t = [cpool.tile([P, BS], F32R, tag=f"xa_t{i}", name=f"xa_t{i}") for i in range(HT)]
            xp_t = [cpool.tile([P, BS], F32R, tag=f"xp_t{i}", name=f"xp_t{i}") for i in range(HT)]
            # Critical path to the first matmuls: xa_t + the first G tiles.
            # Spread issues across engine queues so they don't serialize
            # on one sequencer.
            engs = [nc.sync, nc.gpsimd, nc.scalar]
            fast = [nc.sync, nc.scalar]
            for i in range(HT):
                fast[i % 2].dma_start(out=xa_t[i][:], in_=xa_t_d.ap()[i * P:(i + 1) * P, :])

            # ---- v vectors broadcast across partitions ----------------
            vrow_a = cpool.tile([1, K], F32, tag="vrow_a")
            vrow_p = cpool.tile([1, K], F32, tag="vrow_p")
            nc.gpsimd.dma_start(out=vrow_a[:], in_=v_ap_d.ap().rearrange("k o -> o k"))
            nc.gpsimd.dma_start(out=vrow_p[:], in_=v_pa_d.ap().rearrange("k o -> o k"))
            vbc_a = cpool.tile([P, K], F32, tag="vbc_a")
            vbc_p = cpool.tile([P, K], F32, tag="vbc_p")
            nc.gpsimd.partition_broadcast(vbc_a[:], vrow_a[:])
            nc.gpsimd.partition_broadcast(vbc_p[:], vrow_p[:])

            ones_f = cpool.tile([P, 2], F32, tag="ones_f")
            nc.vector.memset(ones_f[:], 1.0)
            ones_t = cpool.tile([P, 2], F32R, tag="ones_t")
            nc.vector.tensor_copy(ones_t[:], ones_f[:])

            def load_rest():
                # non-critical loads (stage 2+) on gpsimd's own queue
                for i in range(HT):
                    nc.gpsimd.dma_start(out=xp_t[i][:], in_=xp_t_d.ap()[i * P:(i + 1) * P, :])
                for i in range(BL):
                    nc.gpsimd.dma_start(out=xa_nat[i][:], in_=xa_nat_d.ap()[i * P:(i + 1) * P, :])
                    nc.gpsimd.dma_start(out=xp_nat[i][:], in_=xp_nat_d.ap()[i * P:(i + 1) * P, :])

            branches = [
                # (G dram, right_t tiles, stage4 rhs (other), residual (self),
                #  vbc, out dram)
                (g_ap_d, xp_t, xp_nat, xa_nat, vbc_a, out_a_d),
                (g_pa_d, xa_t, xa_nat, xp_nat, vbc_p, out_p_d),
            ]

            for br, (g_d, right_t, nat_other, nat_self, vbc, out_d) in enumerate(branches):
                # ---- stage 1: projT2[g, (b,k,s)] = G.T @ leftT ----------
                # left is always the aspect tensor (faithful to source).
                projT2 = [projpool.tile([P, K, BL, S], F32R, tag=f"projT2_{gt}", name=f"projT2_{gt}")
                          for gt in range(HT)]
                # G loads: per-h tiles so each DMA moves 2KB/14KB contiguous
                # bursts per partition row. k=0 is a separate small tile so the
                # first matmul group doesn't wait on the 7MB bulk.
                PIECES = [(1, 3), (3, 5), (5, 7), (7, 8)]  # [k0, k1) ranges
                gk0 = [gk0p.tile([P, H], F32R, tag=f"gk0_{h}", name=f"gk0_{h}")
                       for h in range(HT)]
                ghb = [[ghbp.tile([P, (k1 - k0) * H], F32R,
                                  tag=f"ghb_{h}_{pi}", name=f"ghb_{h}_{pi}",
                                  bufs=2 if pi == 0 else 1)
                        for pi, (k0, k1) in enumerate(PIECES)]
                       for h in range(HT)]
                for h in range(HT):
                    fast[h % 2].dma_start(
                        out=gk0[h][:], in_=g_d.ap()[h * P:(h + 1) * P, 0:H])
                for pi, (k0, k1) in enumerate(PIECES):
                    for h in range(HT):
                        # last piece rides the otherwise-idle gpsimd queue
                        eng = nc.gpsimd if pi == 3 else fast[(h + pi) % 2]
                        eng.dma_start(
                            out=ghb[h][pi][:],
                            in_=g_d.ap()[h * P:(h + 1) * P, k0 * H:k1 * H])
                for k in range(K):
                    for gt in range(HT):
                        acc = ps_mm.tile([P, BL, S], F32, tag="mmacc", name="acc")
                        for h in range(HT):
                            if k == 0:
                                lhsT = gk0[h][:, gt * P:(gt + 1) * P]
                            else:
                                pi = (k - 1) // 2
                                off = (k - PIECES[pi][0]) * H + gt * P
                                lhsT = ghb[h][pi][:, off:off + P]
                            nc.tensor.matmul(
                                acc[:], lhsT, xa_t[h][:],
                                start=(h == 0), stop=(h == HT - 1))
                        # scalar's stream is busy issuing DMA descriptors for
                        # the first k's — route those evacuations to vector
                        if k < 2 or (k * HT + gt) % 2 == 1:
                            nc.vector.tensor_copy(projT2[gt][:, k, :, :], acc[:])
                        else:
                            nc.scalar.copy(projT2[gt][:, k, :, :], acc[:])

                if br == 0:
                    load_rest()

                # ---- stages 2-4 per batch -------------------------------
                NCK = 2            # k-chunks per batch
                KC = K // NCK      # k's per chunk (4)
                for b in range(BL):
                    th = []
                    for ck in range(NCK):
                        acc2 = ps_mm.tile([P, KC, S], F32, tag="mmacc", name="acc2")
                        for gi in range(HT):
                            nc.tensor.matmul(
                                acc2[:],
                                right_t[gi][:, b * S:(b + 1) * S],
                                projT2[gi][:, ck * KC:(ck + 1) * KC, b, :],
                                start=(gi == 0), stop=(gi == HT - 1))
                        t_sb = work.tile([P, KC, S], F32, tag="tanh", bufs=4)
                        nc.scalar.activation(t_sb[:], acc2[:],
                                             mybir.ActivationFunctionType.Tanh)
                        th.append(t_sb)
                    # weighted sum over k: scoreT[t,s] = sum_k v_k * tanh_k
                    # (two independent chains to shorten the serial path)
                    sca = work.tile([P, S], F32, tag="score_a")
                    scb = work.tile([P, S], F32, tag="score_b")
                    nc.vector.tensor_scalar_mul(sca[:], th[0][:, 0, :], vbc[:, 0:1])
                    nc.vector.tensor_scalar_mul(scb[:], th[1][:, 0, :], vbc[:, KC:KC + 1])
                    for j in range(1, KC):
                        nc.vector.scalar_tensor_tensor(
                            sca[:], th[0][:, j, :], vbc[:, j:j + 1],
                            sca[:], mybir.AluOpType.mult, mybir.AluOpType.add)
                        nc.vector.scalar_tensor_tensor(
                            scb[:], th[1][:, j, :], vbc[:, KC + j:KC + j + 1],
                            scb[:], mybir.AluOpType.mult, mybir.AluOpType.add)
                    sc = work.tile([P, S], F32, tag="score")
                    nc.vector.tensor_tensor(sc[:], sca[:], scb[:], mybir.AluOpType.add)
                    # E_T = exp(scoreT)   (|score| <= sum|v| so no overflow)
                    e_t = work.tile([P, S], F32R, tag="e_t")
                    nc.scalar.activation(e_t[:], sc[:],
                                         mybir.ActivationFunctionType.Exp)
                    # Z[s] = sum_t E_T[t,s]  via ones-matmul
                    zp = ps_z.tile([P, 2], F32, tag="z")
                    nc.tensor.matmul(zp[:], e_t[:], ones_t[:], start=True, stop=True)
                    rz = work.tile([P, 1], F32, tag="rz")
                    nc.vector.reciprocal(rz[:], zp[:, 0:1])
                    # out = self + (E_T.T @ other) / Z
                    rp = ps_o.tile([P, H], F32, tag="raw")
                    nc.tensor.matmul(rp[:], e_t[:], nat_other[b][:],
                                     start=True, stop=True)
                    ot = work.tile([P, H], F32, tag="out")
                    for half, eng in ((0, nc.sync), (1, nc.scalar)):
                        lo, hi = half * (H // 2), (half + 1) * (H // 2)
                        nc.vector.scalar_tensor_tensor(
                            ot[:, lo:hi], rp[:, lo:hi], rz[:, 0:1],
                            nat_self[b][:, lo:hi].bitcast(F32),
                            mybir.AluOpType.mult, mybir.AluOpType.add)
                        eng.dma_start(out=out_d.ap()[b * P:(b + 1) * P, lo:hi],
                                      in_=ot[:, lo:hi])

    nc.compile()
    return nc


def _get_nc():
    if "nc" not in _cache:
        _cache["nc"] = _build()
    return _cache["nc"]


def _prep_in_maps(aspect_hidden, polarity_hidden, G_aspect_polarity,
                  G_polarity_aspect, G_vector_aspect, G_vector_polarity):
    f = np.float32
    a = np.ascontiguousarray(aspect_hidden, dtype=f)
    p = np.ascontiguousarray(polarity_hidden, dtype=f)
    g_ap = np.ascontiguousarray(G_aspect_polarity, dtype=f).reshape(H, KG)
    g_pa = np.ascontiguousarray(G_polarity_aspect, dtype=f).reshape(H, KG)
    v_ap = np.ascontiguousarray(G_vector_aspect, dtype=f)
    v_pa = np.ascontiguousarray(G_vector_polarity, dtype=f)

    in_maps = []
    for c in range(NCORES):
        a_loc = a[c * BL:(c + 1) * BL].reshape(BS, H)
        p_loc = p[c * BL:(c + 1) * BL].reshape(BS, H)
        in_maps.append({
            "xa_nat": a_loc,
            "xp_nat": p_loc,
            "xa_t": np.ascontiguousarray(a_loc.T),
            "xp_t": np.ascontiguousarray(p_loc.T),
            "g_ap": g_ap,
            "g_pa": g_pa,
            "v_ap": v_ap,
            "v_pa": v_pa,
        })
    return in_maps


def kernel(aspect_hidden, polarity_hidden, G_aspect_polarity,
           G_polarity_aspect, G_vector_aspect, G_vector_polarity):
    nc = _get_nc()
    in_maps = _prep_in_maps(aspect_hidden, polarity_hidden, G_aspect_polarity,
                            G_polarity_aspect, G_vector_aspect,
                            G_vector_polarity)
    res = run_bass_kernel_spmd(
        nc, in_maps, core_ids=list(range(NCORES)),
        trace=bool(os.environ.get("KERNEL_TRACE")))
    _cache["last_results"] = res

    out_a = np.empty((B, S, H), np.float32)
    out_p = np.empty((B, S, H), np.float32)
    for c in range(NCORES):
        out_a[c * BL:(c + 1) * BL] = res.results[c]["out_a"].reshape(BL, S, H)
        out_p[c * BL:(c + 1) * BL] = res.results[c]["out_p"].reshape(BL, S, H)
    return (out_a, out_p)



# revision 9
# speedup vs baseline: 1.0801x; 1.0801x over previous
"""CrossSharedUnit Trainium2 kernel — 8-core data-parallel over batch.

Reference computation (per batch b, S=128 tokens, H=512 hidden, K=8):
  proj[b,s,k,g] = sum_h left[b,s,h] * G[h,k,g]
  raw[b,s,t,k]  = tanh(sum_g proj[b,s,k,g] * right[b,t,g])
  score[b,s,t]  = sum_k raw[b,s,t,k] * v[k]
  attn          = softmax(score, axis=t)
  out           = self + attn @ other_hidden
for two branches (aspect: left=aspect, right=polarity; polarity: left=aspect,
right=aspect — faithful to the source which uses aspect on BOTH sides).

Sharding: batch B=32 split 4-per-core across 8 cores; G tensors replicated.
No collectives.

Schedule: the PE (tensor engine) is the bottleneck (~78us of fp32r matmul),
so the program is one continuous PE stream:
  warmup | br0-s1 (+ br0-s2-ck0 spliced at k=3) | br0-s2-ck1
         | br1-s1 (+ br1-s2-ck0 splice + br0 z/out mms interleaved)
         | br1-s2-ck1 | br1 z/out
with softmax chains on vector/gpsimd/scalar underneath the next phase's
matmuls. All input DMA issues live on the sync queue in exact consumption
order (plus two tiny v-loads on gpsimd), so a blocked G prefetch (bufs=1
buffer recycling between branches) can never head-of-line-block a PSUM
evacuation; output stores ride the sync queue after the loads are done.

All matmuls are float32r (TF32-like, 1 row/cycle at free>=256). Softmax
needs no max-subtraction: |score| <= sum|v_k| so exp() cannot overflow in
fp32. The softmax division is deferred through the attention matmul:
out = self + (E @ other) / Z with Z from a ones-matmul.
"""

import os
import sys

sys.path.insert(0, "/opt/trn_rl_repo")

import numpy as np

from concourse import bacc, mybir, tile
from concourse.bass_utils import run_bass_kernel_spmd

B, S, H, K = 32, 128, 512, 8
NCORES = 8
BL = B // NCORES          # batches per core
BS = BL * S               # rows per core (512)
P = 128                   # partitions
HT = H // P               # h partition-tiles (4)
KG = K * H                # flattened (k,g) axis (4096)
KC = K // 2               # k's per stage-2 chunk (4)
F32 = mybir.dt.float32
F32R = mybir.dt.float32r

_cache = {}


def _build():
    """Build + compile the per-core Bass program (same program on all cores)."""
    nc = bacc.Bacc("TRN2", target_bir_lowering=False, debug=False,
                   num_devices=NCORES)

    xa_t_d = nc.dram_tensor("xa_t", [H, BS], F32R, kind="ExternalInput")
    xp_t_d = nc.dram_tensor("xp_t", [H, BS], F32R, kind="ExternalInput")
    xa_nat_d = nc.dram_tensor("xa_nat", [BS, H], F32R, kind="ExternalInput")
    xp_nat_d = nc.dram_tensor("xp_nat", [BS, H], F32R, kind="ExternalInput")
    g_ap_d = nc.dram_tensor("g_ap", [H, KG], F32R, kind="ExternalInput")
    g_pa_d = nc.dram_tensor("g_pa", [H, KG], F32R, kind="ExternalInput")
    v_ap_d = nc.dram_tensor("v_ap", [K, 1], F32, kind="ExternalInput")
    v_pa_d = nc.dram_tensor("v_pa", [K, 1], F32, kind="ExternalInput")
    out_a_d = nc.dram_tensor("out_a", [BS, H], F32, kind="ExternalOutput")
    out_p_d = nc.dram_tensor("out_p", [BS, H], F32, kind="ExternalOutput")

    Tanh = mybir.ActivationFunctionType.Tanh
    Exp = mybir.ActivationFunctionType.Exp
    MULT = mybir.AluOpType.mult
    ADD = mybir.AluOpType.add

    with tile.TileContext(nc) as tc:
        with (
            tc.tile_pool(name="const", bufs=1) as cpool,
            tc.tile_pool(name="g", bufs=1) as gpool,
            tc.tile_pool(name="proj", bufs=1) as projpool,
            tc.tile_pool(name="work", bufs=2) as work,
            tc.tile_pool(name="ps_acc", bufs=4, space="PSUM") as ps_acc,
            tc.tile_pool(name="ps_o", bufs=2, space="PSUM") as ps_o,
            tc.tile_pool(name="ps_z", bufs=2, space="PSUM") as ps_z,
        ):
            # ---- constants + warmup weights (vector queue) --------------
            wm = cpool.tile([P, P], F32R, tag="wm")
            nc.vector.memset(wm[:].bitcast(F32), 0.0)
            ones_t = cpool.tile([P, 2], F32R, tag="ones_t")
            nc.vector.memset(ones_t[:].bitcast(F32), 1.0)

            # ---- persistent activations --------------------------------
            xa_t0 = cpool.tile([P, 1, BS], F32R, tag="xa_t0")
            xa_tb = cpool.tile([P, HT - 1, BS], F32R, tag="xa_tb")
            xp_t = cpool.tile([P, HT, BS], F32R, tag="xp_t")
            xa_nat = cpool.tile([P, BL, H], F32R, tag="xa_nat")
            xp_nat = cpool.tile([P, BL, H], F32R, tag="xp_nat")

            vrow_a = cpool.tile([1, K], F32, tag="vrow_a")
            vrow_p = cpool.tile([1, K], F32, tag="vrow_p")
            nc.gpsimd.dma_start(out=vrow_a[:], in_=v_ap_d.ap().rearrange("k o -> o k"))
            nc.gpsimd.dma_start(out=vrow_p[:], in_=v_pa_d.ap().rearrange("k o -> o k"))
            vbc_a = cpool.tile([P, K], F32, tag="vbc_a")
            vbc_p = cpool.tile([P, K], F32, tag="vbc_p")
            nc.gpsimd.partition_broadcast(vbc_a[:], vrow_a[:])
            nc.gpsimd.partition_broadcast(vbc_p[:], vrow_p[:])

            # ---- G piece tiles (shared bufs=1 between branches) ---------
            g_k0h0 = gpool.tile([P, 1, H], F32R, tag="g_k0h0")
            g_k0hb = gpool.tile([P, HT - 1, H], F32R, tag="g_k0hb")
            g_k1 = gpool.tile([P, HT, H], F32R, tag="g_k1")
            g_bulk = [gpool.tile([P, HT, 2 * H], F32R, tag=f"g_b{i}",
                                 name=f"g_b{i}")
                      for i in range(3)]

            # ---- the loader: every input DMA on sync, consumption order -
            ra = xa_t_d.ap().rearrange("(ht p) bs -> p ht bs", p=P)
            nc.sync.dma_start(out=xa_t0[:], in_=ra[:, 0:1, :])

            def load_g_head(g_d):
                r = g_d.ap().rearrange("(ht p) kg -> p ht kg", p=P)
                nc.sync.dma_start(out=g_k0h0[:], in_=r[:, 0:1, 0:H])
                nc.sync.dma_start(out=g_k0hb[:], in_=r[:, 1:HT, 0:H])
                nc.sync.dma_start(out=g_k1[:], in_=r[:, :, H:2 * H])

            def load_g_bulk(g_d):
                r = g_d.ap().rearrange("(ht p) kg -> p ht kg", p=P)
                for i in range(3):
                    nc.sync.dma_start(
                        out=g_bulk[i][:],
                        in_=r[:, :, (2 + 2 * i) * H:(4 + 2 * i) * H])

            load_g_head(g_ap_d)
            nc.sync.dma_start(out=xa_tb[:], in_=ra[:, 1:HT, :])
            nc.sync.dma_start(
                out=xp_t[:], in_=xp_t_d.ap().rearrange("(ht p) bs -> p ht bs", p=P))
            load_g_bulk(g_ap_d)
            nc.sync.dma_start(
                out=xp_nat[:], in_=xp_nat_d.ap().rearrange("(b p) h -> p b h", p=P))
            # (g_pa loads are emitted after br0-s1 so the WAR deps pick up
            #  br0's reads; xa_nat after those.)

            def g_lhsT(k, h, gt):
                gs = slice(gt * P, (gt + 1) * P)
                if k == 0:
                    if h == 0:
                        return g_k0h0[:, 0, gs]
                    return g_k0hb[:, h - 1, gs]
                if k == 1:
                    return g_k1[:, h, gs]
                piece = g_bulk[(k - 2) // 2]
                off = ((k - 2) % 2) * H + gt * P
                return piece[:, h, off:off + P]

            def xa_rhs(h):
                return xa_t0[:, 0, :] if h == 0 else xa_tb[:, h - 1, :]

            def xa_lhsT(gi, b):
                bs = slice(b * S, (b + 1) * S)
                return xa_t0[:, 0, bs] if gi == 0 else xa_tb[:, gi - 1, bs]

            def xp_lhsT(gi, b):
                return xp_t[:, gi, b * S:(b + 1) * S]

            # projT2[gt][g_part, k, b, s] — stage-1 output, stage-2 rhs.
            projT2 = [projpool.tile([P, K, BL, S], F32R, tag=f"projT2_{gt}",
                                    name=f"projT2_{gt}")
                      for gt in range(HT)]

            evac_state = [0]

            def evac(dst, src):
                # ping-pong PSUM evacuations between vector and scalar
                if evac_state[0] % 2 == 0:
                    nc.vector.tensor_copy(dst, src)
                else:
                    nc.scalar.copy(dst, src)
                evac_state[0] += 1

            # ---- PE warmup: get the p-state ramp going during DMA lead-in
            for w in range(6):
                acc = ps_acc.tile([P, BL, S], F32, tag="acc", name=f"warm{w}")
                nc.tensor.matmul(acc[:, 0, :], wm[:, 0:P], wm[:],
                                 start=True, stop=True)

            def stage1(br):
                # k0 h-outer with 4 open accumulators: first matmuls need
                # only g_k0h0 + xa_t0 (512KB total).
                accs = [ps_acc.tile([P, BL, S], F32, tag="acc",
                                    name=f"s1a{br}k0g{gt}")
                        for gt in range(HT)]
                for h in range(HT):
                    for gt in range(HT):
                        nc.tensor.matmul(
                            accs[gt][:], g_lhsT(0, h, gt), xa_rhs(h),
                            start=(h == 0), stop=(h == HT - 1),
                            skip_group_check=True)
                for gt in range(HT):
                    evac(projT2[gt][:, 0, :, :], accs[gt][:])
                for k in range(1, K):
                    for gt in range(HT):
                        acc = ps_acc.tile([P, BL, S], F32, tag="acc",
                                          name=f"s1a{br}k{k}g{gt}")
                        for h in range(HT):
                            nc.tensor.matmul(
                                acc[:], g_lhsT(k, h, gt), xa_rhs(h),
                                start=(h == 0), stop=(h == HT - 1))
                        evac(projT2[gt][:, k, :, :], acc[:])
                    yield k

            # th_all[t_part, k, b, s]: tanh(stage-2) output, both branches
            # (WAR-recycled). Score ops slice [:, j, :, :] batched over b.
            th_all = work.tile([P, K, BL, S], F32, tag="th", bufs=1)

            def stage2_ck(br, lhsT_of, ck):
                # raw[t, k, s] = tanh(sum_g right[t,g] proj[g,k,s]) per batch
                for b in range(BL):
                    acc2 = ps_acc.tile([P, KC, S], F32, tag="acc",
                                       name=f"s2a{br}b{b}c{ck}")
                    for gi in range(HT):
                        nc.tensor.matmul(
                            acc2[:],
                            lhsT_of(gi, b),
                            projT2[gi][:, ck * KC:(ck + 1) * KC, b, :],
                            start=(gi == 0), stop=(gi == HT - 1))
                    nc.scalar.activation(
                        th_all[:, ck * KC:(ck + 1) * KC, b, :], acc2[:], Tanh)

            def sca_all(vbc):
                # first-half score partial, batched over all 4 batches
                sca = work.tile([P, BL, S], F32, tag="sca")
                nc.vector.tensor_scalar_mul(sca[:], th_all[:, 0, :, :],
                                            vbc[:, 0:1])
                for j in range(1, KC):
                    nc.vector.scalar_tensor_tensor(
                        sca[:], th_all[:, j, :, :], vbc[:, j:j + 1], sca[:],
                        MULT, ADD)
                return sca

            def zout(br, b, e_t, nat_other, nat_self, out_d):
                # out = self + (E_T.T @ other) / Z, Z via ones-matmul.
                zp = ps_z.tile([P, 2], F32, tag="z", name=f"z{br}b{b}")
                nc.tensor.matmul(zp[:], e_t[:, b, :], ones_t[:],
                                 start=True, stop=True)
                rz = work.tile([P, 1], F32, tag="rz", bufs=4)
                nc.vector.reciprocal(rz[:], zp[:, 0:1])
                rp = ps_o.tile([P, H], F32, tag="o", name=f"o{br}b{b}")
                nc.tensor.matmul(rp[:], e_t[:, b, :], nat_other[:, b, :],
                                 start=True, stop=True)
                ot = work.tile([P, H], F32, tag="ot", bufs=2)
                nc.vector.scalar_tensor_tensor(
                    ot[:], rp[:], rz[:, 0:1], nat_self[:, b, :].bitcast(F32),
                    MULT, ADD)
                nc.sync.dma_start(out=out_d.ap()[b * P:(b + 1) * P, :],
                                  in_=ot[:])

            e_t0 = work.tile([P, BL, S], F32R, tag="e0", bufs=1)
            e_t1 = work.tile([P, BL, S], F32R, tag="e1", bufs=1)

            # ================= branch 0 (aspect) ========================
            for k in stage1(0):
                if k == 3:
                    stage2_ck(0, xp_lhsT, 0)
                    sca0 = sca_all(vbc_a)
            load_g_head(g_pa_d)     # prefetch; WAR-gated on br0-s1 reads
            nc.sync.dma_start(
                out=xa_nat[:], in_=xa_nat_d.ap().rearrange("(b p) h -> p b h", p=P))
            load_g_bulk(g_pa_d)
            stage2_ck(0, xp_lhsT, 1)
            # batched second half + exp (runs under br1-s1)
            scb0 = work.tile([P, BL, S], F32, tag="scb", bufs=1)
            nc.vector.tensor_scalar_mul(scb0[:], th_all[:, KC, :, :],
                                        vbc_a[:, KC:KC + 1])
            for j in range(1, KC):
                nc.vector.scalar_tensor_tensor(
                    scb0[:], th_all[:, KC + j, :, :],
                    vbc_a[:, KC + j:KC + j + 1], scb0[:], MULT, ADD)
            sc0 = work.tile([P, BL, S], F32, tag="sc", bufs=1)
            nc.vector.tensor_tensor(sc0[:], sca0[:], scb0[:], ADD)
            nc.scalar.activation(e_t0[:], sc0[:], Exp)

            # ================= branch 1 (polarity) ======================
            # br1 stage 1 with br1-s2-ck0 spliced at k=3 and br0's z/out
            # matmuls interleaved so the PE never waits on softmax chains.
            zo = 0
            for k in stage1(1):
                if k == 3:
                    stage2_ck(1, xa_lhsT, 0)
                    sca1 = sca_all(vbc_p)
                elif k in (2, 4, 5, 6):
                    zout(0, zo, e_t0, xp_nat, xa_nat, out_a_d)
                    zo += 1
            stage2_ck(1, xa_lhsT, 1)

            # Tail: pair-batched second-half chains on vector; the
            # scale+residual combine rides scalar (act-Copy-scale) + gpsimd
            # (tensor add) so no engine saturates after the last tanh.
            Copy = mybir.ActivationFunctionType.Copy
            for pr in range(2):
                bs2 = slice(2 * pr, 2 * pr + 2)
                scb = work.tile([P, 2, S], F32, tag=f"scb1_{pr}", bufs=1)
                nc.vector.tensor_scalar_mul(scb[:], th_all[:, KC, bs2, :],
                                            vbc_p[:, KC:KC + 1])
                for j in range(1, KC):
                    nc.vector.scalar_tensor_tensor(
                        scb[:], th_all[:, KC + j, bs2, :],
                        vbc_p[:, KC + j:KC + j + 1], scb[:], MULT, ADD)
                sc = work.tile([P, 2, S], F32, tag=f"sc1_{pr}", bufs=1)
                nc.vector.tensor_tensor(sc[:], sca1[:, bs2, :], scb[:], ADD)
                nc.scalar.activation(e_t1[:, bs2, :], sc[:], Exp)
                for b in (2 * pr, 2 * pr + 1):
                    zp = ps_z.tile([P, 2], F32, tag="z", name=f"z1b{b}")
                    nc.tensor.matmul(zp[:], e_t1[:, b, :], ones_t[:],
                                     start=True, stop=True)
                    rz = work.tile([P, 1], F32, tag="rz", bufs=4)
                    nc.vector.reciprocal(rz[:], zp[:, 0:1])
                    rp = ps_o.tile([P, H], F32, tag="o", name=f"o1b{b}")
                    nc.tensor.matmul(rp[:], e_t1[:, b, :], xa_nat[:, b, :],
                                     start=True, stop=True)
                    ots = work.tile([P, H], F32, tag="ots", bufs=2)
                    nc.scalar.activation(ots[:], rp[:], Copy,
                                         scale=rz[:, 0:1])
                    otf = work.tile([P, H], F32, tag="otf", bufs=2)
                    nc.gpsimd.tensor_tensor(
                        otf[:], ots[:], xp_nat[:, b, :].bitcast(F32), ADD)
                    nc.sync.dma_start(
                        out=out_p_d.ap()[b * P:(b + 1) * P, :], in_=otf[:])

    nc.compile()
    return nc


def _get_nc():
    if "nc" not in _cache:
        _cache["nc"] = _build()
    return _cache["nc"]


def _prep_in_maps(aspect_hidden, polarity_hidden, G_aspect_polarity,
                  G_polarity_aspect, G_vector_aspect, G_vector_polarity):
    f = np.float32
    a = np.ascontiguousarray(aspect_hidden, dtype=f)
    p = np.ascontiguousarray(polarity_hidden, dtype=f)
    g_ap = np.ascontiguousarray(G_aspect_polarity, dtype=f).reshape(H, KG)
    g_pa = np.ascontiguousarray(G_polarity_aspect, dtype=f).reshape(H, KG)
    v_ap = np.ascontiguousarray(G_vector_aspect, dtype=f)
    v_pa = np.ascontiguousarray(G_vector_polarity, dtype=f)

    in_maps = []
    for c in range(NCORES):
        a_loc = a[c * BL:(c + 1) * BL].reshape(BS, H)
        p_loc = p[c * BL:(c + 1) * BL].reshape(BS, H)
        in_maps.append({
            "xa_t": np.ascontiguousarray(a_loc.T),
            "xp_t": np.ascontiguousarray(p_loc.T),
            "xa_nat": a_loc,
            "xp_nat": p_loc,
            "g_ap": g_ap,
            "g_pa": g_pa,
            "v_ap": v_ap,
            "v_pa": v_pa,
        })
    return in_maps


def kernel(aspect_hidden, polarity_hidden, G_aspect_polarity,
           G_polarity_aspect, G_vector_aspect, G_vector_polarity):
    nc = _get_nc()
    in_maps = _prep_in_maps(aspect_hidden, polarity_hidden, G_aspect_polarity,
                            G_polarity_aspect, G_vector_aspect,
                            G_vector_polarity)
    res = run_bass_kernel_spmd(
        nc, in_maps, core_ids=list(range(NCORES)),
        trace=bool(os.environ.get("KERNEL_TRACE")))
    _cache["last_results"] = res

    out_a = np.empty((B, S, H), np.float32)
    out_p = np.empty((B, S, H), np.float32)
    for c in range(NCORES):
        out_a[c * BL:(c + 1) * BL] = res.results[c]["out_a"].reshape(BL, S, H)
        out_p[c * BL:(c + 1) * BL] = res.results[c]["out_p"].reshape(BL, S, H)
    return (out_a, out_p)


# revision 10
# speedup vs baseline: 1.0854x; 1.0049x over previous
"""CrossSharedUnit Trainium2 kernel — 8-core data-parallel over batch.

Reference computation (per batch b, S=128 tokens, H=512 hidden, K=8):
  proj[b,s,k,g] = sum_h left[b,s,h] * G[h,k,g]
  raw[b,s,t,k]  = tanh(sum_g proj[b,s,k,g] * right[b,t,g])
  score[b,s,t]  = sum_k raw[b,s,t,k] * v[k]
  attn          = softmax(score, axis=t)
  out           = self + attn @ other_hidden
for two branches (aspect: left=aspect, right=polarity; polarity: left=aspect,
right=aspect — faithful to the source which uses aspect on BOTH sides).

Sharding: batch B=32 split 4-per-core across 8 cores; G tensors replicated.
No collectives.

Schedule: the PE (tensor engine) is the bottleneck (~78us of fp32r matmul),
so the program is one continuous PE stream:
  warmup | br0-s1 (+ br0-s2-ck0 spliced at k=3) | br0-s2-ck1
         | br1-s1 (+ br1-s2-ck0 splice + br0 z/out mms interleaved)
         | br1-s2-ck1 | br1 z/out
with softmax chains on vector/gpsimd/scalar underneath the next phase's
matmuls. All input DMA issues live on the sync queue in exact consumption
order (plus two tiny v-loads on gpsimd), so a blocked G prefetch (bufs=1
buffer recycling between branches) can never head-of-line-block a PSUM
evacuation; output stores ride the sync queue after the loads are done.

All matmuls are float32r (TF32-like, 1 row/cycle at free>=256). Softmax
needs no max-subtraction: |score| <= sum|v_k| so exp() cannot overflow in
fp32. The softmax division is deferred through the attention matmul:
out = self + (E @ other) / Z with Z from a ones-matmul.
"""

import os
import sys

sys.path.insert(0, "/opt/trn_rl_repo")

import numpy as np

from concourse import bacc, mybir, tile
from concourse.bass_utils import run_bass_kernel_spmd

B, S, H, K = 32, 128, 512, 8
NCORES = 8
BL = B // NCORES          # batches per core
BS = BL * S               # rows per core (512)
P = 128                   # partitions
HT = H // P               # h partition-tiles (4)
KG = K * H                # flattened (k,g) axis (4096)
KC = K // 2               # k's per stage-2 chunk (4)
F32 = mybir.dt.float32
F32R = mybir.dt.float32r

_cache = {}


def _build():
    """Build + compile the per-core Bass program (same program on all cores)."""
    nc = bacc.Bacc("TRN2", target_bir_lowering=False, debug=False,
                   num_devices=NCORES)

    xa_t_d = nc.dram_tensor("xa_t", [P, HT * BS], F32R, kind="ExternalInput")
    xp_t_d = nc.dram_tensor("xp_t", [P, HT * BS], F32R, kind="ExternalInput")
    xa_nat_d = nc.dram_tensor("xa_nat", [P, BL * H], F32R, kind="ExternalInput")
    xp_nat_d = nc.dram_tensor("xp_nat", [P, BL * H], F32R, kind="ExternalInput")
    g_ap_d = nc.dram_tensor("g_ap", [P, HT * KG], F32R, kind="ExternalInput")
    g_pa_d = nc.dram_tensor("g_pa", [P, HT * KG], F32R, kind="ExternalInput")
    v_ap_d = nc.dram_tensor("v_ap", [K, 1], F32, kind="ExternalInput")
    v_pa_d = nc.dram_tensor("v_pa", [K, 1], F32, kind="ExternalInput")
    out_a_d = nc.dram_tensor("out_a", [BS, H], F32, kind="ExternalOutput")
    out_p_d = nc.dram_tensor("out_p", [BS, H], F32, kind="ExternalOutput")

    Tanh = mybir.ActivationFunctionType.Tanh
    Exp = mybir.ActivationFunctionType.Exp
    MULT = mybir.AluOpType.mult
    ADD = mybir.AluOpType.add

    with tile.TileContext(nc) as tc:
        with (
            tc.tile_pool(name="const", bufs=1) as cpool,
            tc.tile_pool(name="g", bufs=1) as gpool,
            tc.tile_pool(name="proj", bufs=1) as projpool,
            tc.tile_pool(name="work", bufs=2) as work,
            tc.tile_pool(name="ps_acc", bufs=4, space="PSUM") as ps_acc,
            tc.tile_pool(name="ps_o", bufs=2, space="PSUM") as ps_o,
            tc.tile_pool(name="ps_z", bufs=2, space="PSUM") as ps_z,
        ):
            # ---- constants + warmup weights (vector queue) --------------
            wm = cpool.tile([P, BS], F32R, tag="wm")
            nc.vector.memset(wm[:].bitcast(F32), 0.0)
            ones_t = cpool.tile([P, 2], F32R, tag="ones_t")
            nc.vector.memset(ones_t[:].bitcast(F32), 1.0)

            # ---- persistent activations --------------------------------
            xa_t0 = cpool.tile([P, BS], F32R, tag="xa_t0")
            xa_tb = cpool.tile([P, (HT - 1) * BS], F32R, tag="xa_tb")
            xp_t = cpool.tile([P, HT * BS], F32R, tag="xp_t")
            xa_nat = cpool.tile([P, BL * H], F32R, tag="xa_nat")
            xp_nat = cpool.tile([P, BL * H], F32R, tag="xp_nat")

            vrow_a = cpool.tile([1, K], F32, tag="vrow_a")
            vrow_p = cpool.tile([1, K], F32, tag="vrow_p")
            nc.gpsimd.dma_start(out=vrow_a[:], in_=v_ap_d.ap().rearrange("k o -> o k"))
            nc.gpsimd.dma_start(out=vrow_p[:], in_=v_pa_d.ap().rearrange("k o -> o k"))
            vbc_a = cpool.tile([P, K], F32, tag="vbc_a")
            vbc_p = cpool.tile([P, K], F32, tag="vbc_p")
            nc.gpsimd.partition_broadcast(vbc_a[:], vrow_a[:])
            nc.gpsimd.partition_broadcast(vbc_p[:], vrow_p[:])

            # ---- G piece tiles (shared bufs=1 between branches) ---------
            g_k0h0 = gpool.tile([P, H], F32R, tag="g_k0h0")
            g_k0hb = gpool.tile([P, (HT - 1) * H], F32R, tag="g_k0hb")
            g_k1 = gpool.tile([P, HT * H], F32R, tag="g_k1")
            g_bulk = [gpool.tile([P, HT * 2 * H], F32R, tag=f"g_b{i}",
                                 name=f"g_b{i}")
                      for i in range(3)]

            # ---- the loader: every input DMA on sync, consumption order.
            # Host arrays are pre-shuffled partition-major so every DMA is
            # 128 descriptors of one big contiguous chunk (cheap to issue).
            GO = [0, H, HT * H, 2 * HT * H, 4 * HT * H, 6 * HT * H]

            def load_g_head(g_d):
                nc.sync.dma_start(out=g_k0h0[:], in_=g_d.ap()[:, GO[0]:GO[1]])
                nc.sync.dma_start(out=g_k0hb[:], in_=g_d.ap()[:, GO[1]:GO[2]])
                nc.sync.dma_start(out=g_k1[:], in_=g_d.ap()[:, GO[2]:GO[3]])

            def load_g_bulk(g_d):
                for i in range(3):
                    nc.sync.dma_start(
                        out=g_bulk[i][:],
                        in_=g_d.ap()[:, GO[3 + i]:GO[3 + i] + 2 * HT * H])

            nc.sync.dma_start(out=xa_t0[:], in_=xa_t_d.ap()[:, 0:BS])
            load_g_head(g_ap_d)
            nc.sync.dma_start(out=xa_tb[:], in_=xa_t_d.ap()[:, BS:HT * BS])
            nc.sync.dma_start(out=xp_t[:], in_=xp_t_d.ap()[:])
            load_g_bulk(g_ap_d)
            nc.sync.dma_start(out=xp_nat[:], in_=xp_nat_d.ap()[:])
            # (g_pa loads are emitted after br0-s1 so the WAR deps pick up
            #  br0's reads; xa_nat after those.)

            def g_lhsT(k, h, gt):
                if k == 0:
                    if h == 0:
                        return g_k0h0[:, gt * P:(gt + 1) * P]
                    o = (h - 1) * H + gt * P
                    return g_k0hb[:, o:o + P]
                if k == 1:
                    o = h * H + gt * P
                    return g_k1[:, o:o + P]
                piece = g_bulk[(k - 2) // 2]
                o = h * 2 * H + ((k - 2) % 2) * H + gt * P
                return piece[:, o:o + P]

            def xa_rhs(h):
                if h == 0:
                    return xa_t0[:]
                return xa_tb[:, (h - 1) * BS:h * BS]

            def xa_lhsT(gi, b):
                if gi == 0:
                    return xa_t0[:, b * S:(b + 1) * S]
                o = (gi - 1) * BS + b * S
                return xa_tb[:, o:o + S]

            def xp_lhsT(gi, b):
                o = gi * BS + b * S
                return xp_t[:, o:o + S]

            # projT2[gt][g_part, k, b, s] — stage-1 output, stage-2 rhs.
            projT2 = [projpool.tile([P, K, BL, S], F32R, tag=f"projT2_{gt}",
                                    name=f"projT2_{gt}")
                      for gt in range(HT)]

            evac_state = [0]

            def evac(dst, src):
                # ping-pong PSUM evacuations between vector and scalar
                if evac_state[0] % 2 == 0:
                    nc.vector.tensor_copy(dst, src)
                else:
                    nc.scalar.copy(dst, src)
                evac_state[0] += 1

            # ---- PE warmup: get the p-state ramp going during DMA lead-in
            for w in range(7):
                acc = ps_acc.tile([P, BL, S], F32, tag="acc", name=f"warm{w}")
                nc.tensor.matmul(acc[:], wm[:, 0:P], wm[:],
                                 start=True, stop=True)

            def stage1(br):
                # k0 h-outer with 4 open accumulators: first matmuls need
                # only g_k0h0 + xa_t0 (512KB total).
                accs = [ps_acc.tile([P, BL, S], F32, tag="acc",
                                    name=f"s1a{br}k0g{gt}")
                        for gt in range(HT)]
                for h in range(HT):
                    for gt in range(HT):
                        nc.tensor.matmul(
                            accs[gt][:], g_lhsT(0, h, gt), xa_rhs(h),
                            start=(h == 0), stop=(h == HT - 1),
                            skip_group_check=True)
                for gt in range(HT):
                    evac(projT2[gt][:, 0, :, :], accs[gt][:])
                for k in range(1, K):
                    for gt in range(HT):
                        acc = ps_acc.tile([P, BL, S], F32, tag="acc",
                                          name=f"s1a{br}k{k}g{gt}")
                        for h in range(HT):
                            nc.tensor.matmul(
                                acc[:], g_lhsT(k, h, gt), xa_rhs(h),
                                start=(h == 0), stop=(h == HT - 1))
                        evac(projT2[gt][:, k, :, :], acc[:])
                    yield k

            # th_all[t_part, k, b, s]: tanh(stage-2) output, both branches
            # (WAR-recycled). Score ops slice [:, j, :, :] batched over b.
            th_all = work.tile([P, K, BL, S], F32, tag="th", bufs=1)

            def stage2_ck(br, lhsT_of, ck):
                # raw[t, k, s] = tanh(sum_g right[t,g] proj[g,k,s]) per batch
                for b in range(BL):
                    acc2 = ps_acc.tile([P, KC, S], F32, tag="acc",
                                       name=f"s2a{br}b{b}c{ck}")
                    for gi in range(HT):
                        nc.tensor.matmul(
                            acc2[:],
                            lhsT_of(gi, b),
                            projT2[gi][:, ck * KC:(ck + 1) * KC, b, :],
                            start=(gi == 0), stop=(gi == HT - 1))
                    nc.scalar.activation(
                        th_all[:, ck * KC:(ck + 1) * KC, b, :], acc2[:], Tanh)

            def sca_all(vbc):
                # first-half score partial, batched over all 4 batches
                sca = work.tile([P, BL, S], F32, tag="sca")
                nc.vector.tensor_scalar_mul(sca[:], th_all[:, 0, :, :],
                                            vbc[:, 0:1])
                for j in range(1, KC):
                    nc.vector.scalar_tensor_tensor(
                        sca[:], th_all[:, j, :, :], vbc[:, j:j + 1], sca[:],
                        MULT, ADD)
                return sca

            def zout(br, b, e_t, nat_other, nat_self, out_d):
                # out = self + (E_T.T @ other) / Z, Z via ones-matmul.
                zp = ps_z.tile([P, 2], F32, tag="z", name=f"z{br}b{b}")
                nc.tensor.matmul(zp[:], e_t[:, b, :], ones_t[:],
                                 start=True, stop=True)
                rz = work.tile([P, 1], F32, tag="rz", bufs=4)
                nc.vector.reciprocal(rz[:], zp[:, 0:1])
                rp = ps_o.tile([P, H], F32, tag="o", name=f"o{br}b{b}")
                nc.tensor.matmul(rp[:], e_t[:, b, :], nat_other[:, b * H:(b + 1) * H],
                                 start=True, stop=True)
                ot = work.tile([P, H], F32, tag="ot", bufs=2)
                nc.vector.scalar_tensor_tensor(
                    ot[:], rp[:], rz[:, 0:1], nat_self[:, b * H:(b + 1) * H].bitcast(F32),
                    MULT, ADD)
                nc.sync.dma_start(out=out_d.ap()[b * P:(b + 1) * P, :],
                                  in_=ot[:])

            e_t0 = work.tile([P, BL, S], F32R, tag="e0", bufs=1)
            e_t1 = work.tile([P, BL, S], F32R, tag="e1", bufs=1)

            # ================= branch 0 (aspect) ========================
            for k in stage1(0):
                if k == 3:
                    stage2_ck(0, xp_lhsT, 0)
                    sca0 = sca_all(vbc_a)
            load_g_head(g_pa_d)     # prefetch; WAR-gated on br0-s1 reads
            nc.sync.dma_start(out=xa_nat[:], in_=xa_nat_d.ap()[:])
            load_g_bulk(g_pa_d)
            stage2_ck(0, xp_lhsT, 1)
            # batched second half + exp (runs under br1-s1)
            scb0 = work.tile([P, BL, S], F32, tag="scb", bufs=1)
            nc.vector.tensor_scalar_mul(scb0[:], th_all[:, KC, :, :],
                                        vbc_a[:, KC:KC + 1])
            for j in range(1, KC):
                nc.vector.scalar_tensor_tensor(
                    scb0[:], th_all[:, KC + j, :, :],
                    vbc_a[:, KC + j:KC + j + 1], scb0[:], MULT, ADD)
            sc0 = work.tile([P, BL, S], F32, tag="sc", bufs=1)
            nc.vector.tensor_tensor(sc0[:], sca0[:], scb0[:], ADD)
            nc.scalar.activation(e_t0[:], sc0[:], Exp)

            # ================= branch 1 (polarity) ======================
            # br1 stage 1 with br1-s2-ck0 spliced at k=3 and br0's z/out
            # matmuls interleaved so the PE never waits on softmax chains.
            zo = 0
            for k in stage1(1):
                if k == 3:
                    stage2_ck(1, xa_lhsT, 0)
                    sca1 = sca_all(vbc_p)
                elif k in (2, 4, 5, 6):
                    zout(0, zo, e_t0, xp_nat, xa_nat, out_a_d)
                    zo += 1
            stage2_ck(1, xa_lhsT, 1)

            # Tail: pair-batched second-half chains on vector; the
            # scale+residual combine rides scalar (act-Copy-scale) + gpsimd
            # (tensor add) so no engine saturates after the last tanh.
            Copy = mybir.ActivationFunctionType.Copy
            for pr in range(2):
                bs2 = slice(2 * pr, 2 * pr + 2)
                scb = work.tile([P, 2, S], F32, tag=f"scb1_{pr}", bufs=1)
                nc.vector.tensor_scalar_mul(scb[:], th_all[:, KC, bs2, :],
                                            vbc_p[:, KC:KC + 1])
                for j in range(1, KC):
                    nc.vector.scalar_tensor_tensor(
                        scb[:], th_all[:, KC + j, bs2, :],
                        vbc_p[:, KC + j:KC + j + 1], scb[:], MULT, ADD)
                sc = work.tile([P, 2, S], F32, tag=f"sc1_{pr}", bufs=1)
                nc.vector.tensor_tensor(sc[:], sca1[:, bs2, :], scb[:], ADD)
                nc.scalar.activation(e_t1[:, bs2, :], sc[:], Exp)
                for b in (2 * pr, 2 * pr + 1):
                    zp = ps_z.tile([P, 2], F32, tag="z", name=f"z1b{b}")
                    nc.tensor.matmul(zp[:], e_t1[:, b, :], ones_t[:],
                                     start=True, stop=True)
                    rz = work.tile([P, 1], F32, tag="rz", bufs=4)
                    nc.vector.reciprocal(rz[:], zp[:, 0:1])
                    rp = ps_o.tile([P, H], F32, tag="o", name=f"o1b{b}")
                    nc.tensor.matmul(rp[:], e_t1[:, b, :], xa_nat[:, b * H:(b + 1) * H],
                                     start=True, stop=True)
                    ots = work.tile([P, H], F32, tag="ots", bufs=2)
                    nc.scalar.activation(ots[:], rp[:], Copy,
                                         scale=rz[:, 0:1])
                    otf = work.tile([P, H], F32, tag="otf", bufs=2)
                    nc.gpsimd.tensor_tensor(
                        otf[:], ots[:], xp_nat[:, b * H:(b + 1) * H].bitcast(F32), ADD)
                    nc.sync.dma_start(
                        out=out_p_d.ap()[b * P:(b + 1) * P, :], in_=otf[:])

    nc.compile()
    return nc


def _get_nc():
    if "nc" not in _cache:
        _cache["nc"] = _build()
    return _cache["nc"]


def _prep_in_maps(aspect_hidden, polarity_hidden, G_aspect_polarity,
                  G_polarity_aspect, G_vector_aspect, G_vector_polarity):
    f = np.float32

    def shuffle_g(g):
        # host-side image of the SBUF G piece tiles, concatenated: each DMA
        # is then one contiguous chunk per partition
        gr = np.asarray(g, dtype=f).reshape(HT, P, K, H)
        pieces = [
            gr[0, :, 0, :].reshape(P, H),
            gr[1:, :, 0, :].transpose(1, 0, 2).reshape(P, (HT - 1) * H),
            gr[:, :, 1, :].transpose(1, 0, 2).reshape(P, HT * H),
        ]
        for i in range(3):
            pieces.append(gr[:, :, 2 + 2 * i:4 + 2 * i, :]
                          .transpose(1, 0, 2, 3).reshape(P, HT * 2 * H))
        return np.ascontiguousarray(np.concatenate(pieces, axis=1))

    def shuffle_t(x_loc):
        # [BS,H] -> transposed partition-major [P, (ht, bs)]
        return np.ascontiguousarray(
            x_loc.T.reshape(HT, P, BS).transpose(1, 0, 2).reshape(P, HT * BS))

    def shuffle_nat(x_loc):
        # [BS,H] -> partition-major [P, (b, h)]
        return np.ascontiguousarray(
            x_loc.reshape(BL, P, H).transpose(1, 0, 2).reshape(P, BL * H))

    a = np.ascontiguousarray(aspect_hidden, dtype=f)
    p = np.ascontiguousarray(polarity_hidden, dtype=f)
    g_ap = shuffle_g(G_aspect_polarity)
    g_pa = shuffle_g(G_polarity_aspect)
    v_ap = np.ascontiguousarray(G_vector_aspect, dtype=f)
    v_pa = np.ascontiguousarray(G_vector_polarity, dtype=f)

    in_maps = []
    for c in range(NCORES):
        a_loc = a[c * BL:(c + 1) * BL].reshape(BS, H)
        p_loc = p[c * BL:(c + 1) * BL].reshape(BS, H)
        in_maps.append({
            "xa_t": shuffle_t(a_loc),
            "xp_t": shuffle_t(p_loc),
            "xa_nat": shuffle_nat(a_loc),
            "xp_nat": shuffle_nat(p_loc),
            "g_ap": g_ap,
            "g_pa": g_pa,
            "v_ap": v_ap,
            "v_pa": v_pa,
        })
    return in_maps


def kernel(aspect_hidden, polarity_hidden, G_aspect_polarity,
           G_polarity_aspect, G_vector_aspect, G_vector_polarity):
    nc = _get_nc()
    in_maps = _prep_in_maps(aspect_hidden, polarity_hidden, G_aspect_polarity,
                            G_polarity_aspect, G_vector_aspect,
                            G_vector_polarity)
    res = run_bass_kernel_spmd(
        nc, in_maps, core_ids=list(range(NCORES)),
        trace=bool(os.environ.get("KERNEL_TRACE")))
    _cache["last_results"] = res

    out_a = np.empty((B, S, H), np.float32)
    out_p = np.empty((B, S, H), np.float32)
    for c in range(NCORES):
        out_a[c * BL:(c + 1) * BL] = res.results[c]["out_a"].reshape(BL, S, H)
        out_p[c * BL:(c + 1) * BL] = res.results[c]["out_p"].reshape(BL, S, H)
    return (out_a, out_p)
